# revision 26
# baseline (speedup 1.0000x reference)
"""Trainium2 Bass kernel for NaiveKHopGraphAttention (lane + edge-slab).

Strategy (no collectives, no device-side gather):
  - Host (integer index work only): sort nodes by degree, group into
    128-node blocks of near-equal degree, assign blocks to (core, slot)
    with SPMD-uniform per-slot tile counts. Tile t of a slot holds the
    t-th edge of each of the block's 128 nodes (lane layout). The host
    ships, per core, the transposed X rows of each lane's dst node
    (XET = X[dst].T, bf16) - pure indexing of the input, no float math.
    Pad lanes point at a zero row.
  - Device phases:
      A: Q projection -> SBUF-resident qx (bf16, node-partition).
      C: per slot, per 4-tile group: PE projects K|V per edge
         (lhsT = slab tile, rhs = [Wk.T|Wv.T]) into PSUM; rotating
         engine copies PSUM->SBUF bf16; DVE qk = K*q_bcast; Pool
         per-head score reduce; Act exp (strided into rhs[:, :, D:]);
         DVE wv = V*ex_bcast; PE identity-matmul accumulates each
         tile's [wv | ex] into a per-slot PSUM [num | den] (lane p is
         node p, so the segment sum is a plain copy-accumulate).
         Dummy lanes contribute ex=1, V=0; the denominator is fixed
         exactly with a host-computed dummy count. Then divide and
         store the attn row block into SBUF.
      D: batched epilogue: LayerNorm1 across all slots in wide DVE ops
         (g1 folded into Wo), per-slot PE transpose + out-projection,
         batched LayerNorm2, single output DMA (bf16; host converts).
"""

import sys

if "/opt/trn_rl_repo" not in sys.path:
    sys.path.insert(0, "/opt/trn_rl_repo")

import ml_dtypes
import numpy as np

BF16NP = ml_dtypes.bfloat16

import concourse.bacc as bacc
import concourse.bass as bass
import concourse.mybir as mybir
import concourse.tile as tile
from concourse.bass_utils import run_bass_kernel_spmd

F32 = mybir.dt.float32
BF16 = mybir.dt.bfloat16
I32 = mybir.dt.int32

NCORES = 8
P = 128
EPS = 1e-5
DEN_GUARD = 1e-30
G = 4        # edge tiles per compute group


# ----------------------------------------------------------------------------
# Host-side preprocessing
# ----------------------------------------------------------------------------

def _schedule(src, dst, n_nodes):
    n_blocks = -(-n_nodes // P)
    n_blocks = -(-n_blocks // NCORES) * NCORES
    n_pad = n_blocks * P
    slots = n_blocks // NCORES
    assert n_pad > n_nodes, "need at least one zero pad node"

    deg = np.bincount(src, minlength=n_pad).astype(np.int64)

    perm = np.argsort(-deg, kind="stable")
    pos = np.empty(n_pad, dtype=np.int64)
    pos[perm] = np.arange(n_pad)
    blk = pos // P
    p_of = pos % P
    j_of = blk // NCORES
    c_of = blk % NCORES

    # per-slot tile count = max degree across the slot's 8 blocks
    deg_blk = deg[perm].reshape(n_blocks, P).max(axis=1)
    stn = deg_blk.reshape(slots, NCORES).max(axis=1)
    stn = np.maximum(stn, 1)
    tile_off = np.zeros(slots + 1, dtype=np.int64)
    np.cumsum(stn, out=tile_off[1:])
    T = int(tile_off[-1])

    # lane dst ids: LID[c, p, tile_off[j]+t] = dst of node's t-th edge
    order = np.argsort(src, kind="stable")
    src_s = src[order]
    dst_s = dst[order]
    cnt = np.bincount(src, minlength=n_pad)
    noff = np.zeros(n_pad + 1, dtype=np.int64)
    np.cumsum(cnt, out=noff[1:])
    rank = np.arange(len(order)) - noff[src_s]

    lid = np.full((NCORES, P, T), n_pad - 1, dtype=np.int64)  # pad: zero row
    cs, ps, js = c_of[src_s], p_of[src_s], j_of[src_s]
    lid[cs, ps, tile_off[js] + rank] = dst_s

    dcnt = np.empty((NCORES, P, slots), dtype=np.float32)
    dcnt[c_of, p_of, j_of] = (stn[j_of] - deg).astype(np.float32)

    nodeids = np.empty((NCORES, slots * P), dtype=np.int64)
    nodeids[c_of, j_of * P + p_of] = np.arange(n_pad)

    return {
        "n_pad": n_pad,
        "slots": slots,
        "T": T,
        "slot_tiles": [int(x) for x in stn],
        "lid": lid,
        "dcnt": dcnt,
        "nodeids": nodeids,
    }


def _prep_inputs(X, attn_window, Wq, bq, Wk, bk, Wv, bv, Wo, bo, g1, b1, g2, b2):
    n_nodes, D = X.shape
    src = np.asarray(attn_window[0]).astype(np.int64)
    dst = np.asarray(attn_window[1]).astype(np.int64)
    sch = _schedule(src, dst, n_nodes)
    n_pad, slots, T = sch["n_pad"], sch["slots"], sch["T"]

    Xp = np.zeros((n_pad, D), dtype=np.float32)
    Xp[:n_nodes] = np.asarray(X, np.float32)
    XTb = np.ascontiguousarray(Xp.T).astype(BF16NP)  # [D, n_pad] bf16

    WoT = np.asarray(Wo, np.float32).T
    Wo2T = np.ascontiguousarray(WoT * np.asarray(g1, np.float32)[:, None])
    BO2 = (np.asarray(b1, np.float32) @ WoT + np.asarray(bo, np.float32))[None, :]

    has_bkv = bool(np.any(np.asarray(bk) != 0) or np.any(np.asarray(bv) != 0))
    has_bq = bool(np.any(np.asarray(bq) != 0))
    has_bo2 = bool(np.any(BO2 != 0))
    has_g2 = bool(np.any(np.asarray(g2) != 1))
    has_b2 = bool(np.any(np.asarray(b2) != 0))
    flags = (has_bkv, has_bq, has_bo2, has_g2, has_b2)

    common = {
        "WKVT": np.ascontiguousarray(
            np.concatenate([np.asarray(Wk, np.float32).T,
                            np.asarray(Wv, np.float32).T], axis=1)
        ).astype(BF16NP),
        "WQT": np.ascontiguousarray(np.asarray(Wq, np.float32).T).astype(BF16NP),
        "WO2T": Wo2T.astype(BF16NP),
        "IDENT": np.eye(P, dtype=np.float32).astype(BF16NP),
    }
    if has_bkv:
        common["BKVR"] = np.broadcast_to(
            np.concatenate([np.asarray(bk, np.float32),
                            np.asarray(bv, np.float32)])[None, :],
            (P, 2 * D)).copy()
    if has_bq:
        common["BQR"] = np.broadcast_to(
            np.asarray(bq, np.float32)[None, :], (P, D)).copy()
    if has_bo2:
        common["BO2R"] = np.broadcast_to(BO2, (P, D)).copy()
    if has_g2:
        common["G2R"] = np.broadcast_to(
            np.asarray(g2, np.float32)[None, :], (P, D)).astype(BF16NP).copy()
    if has_b2:
        common["B2R"] = np.broadcast_to(
            np.asarray(b2, np.float32)[None, :], (P, D)).astype(BF16NP).copy()

    in_maps = []
    for c in range(NCORES):
        m = dict(common)
        m["XTQ"] = np.ascontiguousarray(
            Xp[sch["nodeids"][c]].T).astype(BF16NP)
        # edge slab: X rows of each lane's dst, transposed [D, T*P].
        # lid[c] is [P, T] (lane p, tile t); lane order in a tile is p.
        lid_flat = sch["lid"][c].T.ravel()               # t-major, then p
        m["XET"] = np.ascontiguousarray(XTb[:, lid_flat])
        m["DCNT"] = np.ascontiguousarray(sch["dcnt"][c])
        in_maps.append(m)
    return sch, in_maps, flags


# ----------------------------------------------------------------------------
# Device kernel
# ----------------------------------------------------------------------------

def _newton_rsqrt(nc, pool, v_ap, width, tag, iters=2):
    """rstd = 1/sqrt(v) on DVE only. v_ap: [P, width] f32 (eps added)."""
    y = pool.tile([P, width], F32, tag=tag + "_y")
    u = pool.tile([P, width], I32, tag=tag + "_u")
    nc.vector.tensor_scalar(
        out=u[:], in0=v_ap.bitcast(I32), scalar1=1, scalar2=None,
        op0=mybir.AluOpType.arith_shift_right)
    nc.vector.tensor_scalar(
        out=y[:].bitcast(I32), in0=u[:], scalar1=0x5F3759DF, scalar2=-1,
        op0=mybir.AluOpType.subtract, op1=mybir.AluOpType.mult)
    t = pool.tile([P, width], F32, tag=tag + "_t")
    for _ in range(iters):
        nc.vector.tensor_mul(t[:], y[:], y[:])
        nc.vector.tensor_mul(t[:], t[:], v_ap)
        nc.vector.tensor_scalar(
            out=t[:], in0=t[:], scalar1=-0.5, scalar2=1.5,
            op0=mybir.AluOpType.mult, op1=mybir.AluOpType.add)
        nc.vector.tensor_mul(y[:], y[:], t[:])
    return y


def build_program(n_pad, slots, T, slot_tiles, D=128, H=8,
                  flags=(False, False, False, False, False)):
    has_bkv, has_bq, has_bo2, has_g2, has_b2 = flags
    HD = D // H
    scale = 1.0 / np.sqrt(HD)
    S = slots * D

    nc = bacc.Bacc("TRN2", target_bir_lowering=False, debug=False,
                   num_devices=NCORES)

    xet = nc.dram_tensor("XET", [D, T * P], BF16, kind="ExternalInput").ap()
    xtq = nc.dram_tensor("XTQ", [D, slots * P], BF16, kind="ExternalInput").ap()
    wkvt = nc.dram_tensor("WKVT", [D, 2 * D], BF16, kind="ExternalInput").ap()
    wqt = nc.dram_tensor("WQT", [D, D], BF16, kind="ExternalInput").ap()
    wo2t = nc.dram_tensor("WO2T", [D, D], BF16, kind="ExternalInput").ap()
    ident_in = nc.dram_tensor("IDENT", [P, P], BF16, kind="ExternalInput").ap()
    dcnt_in = nc.dram_tensor("DCNT", [P, slots], F32, kind="ExternalInput").ap()
    if has_bkv:
        bkvr = nc.dram_tensor("BKVR", [P, 2 * D], F32, kind="ExternalInput").ap()
    if has_bq:
        bqr = nc.dram_tensor("BQR", [P, D], F32, kind="ExternalInput").ap()
    if has_bo2:
        bo2r = nc.dram_tensor("BO2R", [P, D], F32, kind="ExternalInput").ap()
    if has_g2:
        g2r = nc.dram_tensor("G2R", [P, D], BF16, kind="ExternalInput").ap()
    if has_b2:
        b2r = nc.dram_tensor("B2R", [P, D], BF16, kind="ExternalInput").ap()
    out = nc.dram_tensor("OUT", [slots * P, D], BF16, kind="ExternalOutput").ap()

    with tile.TileContext(nc) as tc:
        with (
            tc.tile_pool(name="consts", bufs=1) as consts,
            tc.tile_pool(name="slab", bufs=2) as slab,
            tc.tile_pool(name="kvsp", bufs=3) as kvsp,
            tc.tile_pool(name="edges", bufs=2) as edges,
            tc.tile_pool(name="sct", bufs=3) as sct,
            tc.tile_pool(name="blk", bufs=2) as blk,
            tc.tile_pool(name="epi", bufs=2) as epi,
            tc.tile_pool(name="mmb", bufs=2, space="PSUM") as mmb,
            tc.tile_pool(name="mmseg", bufs=2, space="PSUM") as mmseg,
            tc.tile_pool(name="mmd", bufs=1, space="PSUM") as mmd,
        ):
            # ---- constants
            c_wkvt = consts.tile([D, 2 * D], BF16, tag="wkvt")
            nc.sync.dma_start(out=c_wkvt[:], in_=wkvt[:])
            c_wqt = consts.tile([D, D], BF16, tag="wqt")
            nc.sync.dma_start(out=c_wqt[:], in_=wqt[:])
            c_wo2t = consts.tile([D, D], BF16, tag="wo2t")
            nc.sync.dma_start(out=c_wo2t[:], in_=wo2t[:])
            c_ident = consts.tile([P, P], BF16, tag="ident")
            nc.sync.dma_start(out=c_ident[:], in_=ident_in[:])
            c_dcnt = consts.tile([P, slots], F32, tag="dcnt")
            nc.sync.dma_start(out=c_dcnt[:], in_=dcnt_in[:])
            if has_bkv:
                c_bkvr = consts.tile([P, 2 * D], F32, tag="bkvr")
                nc.sync.dma_start(out=c_bkvr[:], in_=bkvr[:])
            if has_bq:
                c_bqr = consts.tile([P, D], F32, tag="bqr")
                nc.sync.dma_start(out=c_bqr[:], in_=bqr[:])
            if has_bo2:
                c_bo2r = consts.tile([P, D], F32, tag="bo2r")
                nc.sync.dma_start(out=c_bo2r[:], in_=bo2r[:])
            if has_g2:
                c_g2 = consts.tile([P, D], BF16, tag="g2")
                nc.sync.dma_start(out=c_g2[:], in_=g2r[:])
            if has_b2:
                c_b2 = consts.tile([P, D], BF16, tag="b2")
                nc.sync.dma_start(out=c_b2[:], in_=b2r[:])
            c_qx = consts.tile([P, slots * D], BF16, tag="qx")

            # ---- Phase C: edge stage per slot (two passes, big fused ops)
            # The epilogue (LN1 -> out-proj -> LN2 -> store) runs in
            # CH-slot chunks interleaved into the slot loop so it hides
            # under edge-phase compute instead of forming a serial tail.
            ST = max(slot_tiles)
            CH = 7
            assert slots % CH == 0
            SC = CH * D

            def layer_norm_chunk(x_flat, out_flat, tagp="ln"):
                x3 = x_flat.rearrange("p (j d) -> p j d", j=CH)
                s1 = epi.tile([P, CH], F32, tag=tagp + "s1")
                nc.vector.tensor_reduce(out=s1[:], in_=x3,
                                        axis=mybir.AxisListType.X,
                                        op=mybir.AluOpType.add)
                nm = epi.tile([P, CH], F32, tag=tagp + "nm")
                nc.vector.tensor_scalar(
                    out=nm[:], in0=s1[:], scalar1=-1.0 / D, scalar2=None,
                    op0=mybir.AluOpType.mult)
                sq = epi.tile([P, SC], BF16, tag=tagp + "sq")
                nc.vector.tensor_tensor(out=sq[:], in0=x_flat, in1=x_flat,
                                        op=mybir.AluOpType.mult)
                s2 = epi.tile([P, CH], F32, tag=tagp + "s2")
                nc.vector.tensor_reduce(
                    out=s2[:], in_=sq[:].rearrange("p (j d) -> p j d", j=CH),
                    axis=mybir.AxisListType.X, op=mybir.AluOpType.add)
                ve = epi.tile([P, CH], F32, tag=tagp + "ve")
                nc.vector.tensor_mul(ve[:], nm[:], nm[:])
                nc.vector.scalar_tensor_tensor(
                    out=ve[:], in0=s2[:], scalar=1.0 / D, in1=ve[:],
                    op0=mybir.AluOpType.mult, op1=mybir.AluOpType.subtract)
                nc.vector.tensor_scalar(
                    out=ve[:], in0=ve[:], scalar1=EPS, scalar2=None,
                    op0=mybir.AluOpType.add)
                rstd = _newton_rsqrt(nc, epi, ve[:], CH, tagp + "r")
                nm0, rs0 = nm[:], rstd[:]
                nm_b = bass.AP(nm0.tensor, nm0.offset,
                               [nm0.ap[0], [1, CH], [0, D]])
                rs_b = bass.AP(rs0.tensor, rs0.offset,
                               [rs0.ap[0], [1, CH], [0, D]])
                xm = epi.tile([P, SC], BF16, tag=tagp + "xm")
                nc.vector.tensor_tensor(
                    out=xm[:].rearrange("p (j d) -> p j d", j=CH),
                    in0=x3, in1=nm_b, op=mybir.AluOpType.add)
                nc.vector.tensor_tensor(
                    out=out_flat.rearrange("p (j d) -> p j d", j=CH),
                    in0=xm[:].rearrange("p (j d) -> p j d", j=CH),
                    in1=rs_b, op=mybir.AluOpType.mult)

            def emit_chunk(j0, attn_ch):
                xh = epi.tile([P, SC], BF16, tag="xh")
                layer_norm_chunk(attn_ch[:], xh[:])
                o2 = epi.tile([P, SC], BF16, tag="o2")
                for jj in range(CH):
                    pst = mmd.tile([P, D], BF16, tag="pd")
                    nc.tensor.transpose(out=pst[:],
                                        in_=xh[:, jj * D:(jj + 1) * D],
                                        identity=c_ident[:])
                    lnt = blk.tile([P, D], BF16, tag="lnt")
                    nc.scalar.copy(lnt[:], pst[:])
                    ps2 = mmd.tile([P, D], F32, tag="pd2")
                    nc.tensor.matmul(out=ps2[:], lhsT=lnt[:], rhs=c_wo2t[:],
                                     start=True, stop=True)
                    if has_bo2:
                        nc.vector.tensor_add(o2[:, jj * D:(jj + 1) * D],
                                             ps2[:], c_bo2r[:])
                    elif jj % 2 == 0:
                        nc.scalar.copy(o2[:, jj * D:(jj + 1) * D], ps2[:])
                    else:
                        nc.vector.tensor_copy(o2[:, jj * D:(jj + 1) * D],
                                              ps2[:])
                fin = epi.tile([P, SC], BF16, tag="xh")  # reuse xh buffer
                layer_norm_chunk(o2[:], fin[:])
                fin_ap = fin[:]
                if has_g2:
                    gg = c_g2[:]
                    g_b = bass.AP(gg.tensor, gg.offset,
                                  [gg.ap[0], [0, CH], [1, D]])
                    fg = epi.tile([P, SC], BF16, tag="lnsq")
                    nc.vector.tensor_tensor(
                        out=fg[:].rearrange("p (j d) -> p j d", j=CH),
                        in0=fin_ap.rearrange("p (j d) -> p j d", j=CH),
                        in1=g_b, op=mybir.AluOpType.mult)
                    fin_ap = fg[:]
                if has_b2:
                    bb = c_b2[:]
                    b_b = bass.AP(bb.tensor, bb.offset,
                                  [bb.ap[0], [0, CH], [1, D]])
                    fb = epi.tile([P, SC], BF16, tag="lnxm")
                    nc.vector.tensor_tensor(
                        out=fb[:].rearrange("p (j d) -> p j d", j=CH),
                        in0=fin_ap.rearrange("p (j d) -> p j d", j=CH),
                        in1=b_b, op=mybir.AluOpType.add)
                    fin_ap = fb[:]
                nc.sync.dma_start(
                    out=out[j0 * P:(j0 + CH) * P, :].rearrange(
                        "(j p) d -> p j d", p=P),
                    in_=fin_ap.rearrange("p (j d) -> p j d", j=CH))

            ti = 0
            attn_ch = None
            for j in range(slots):
                stn = slot_tiles[j]
                # Q projection for this slot (interleaved with edge work)
                xq = slab.tile([D, P], BF16, tag="xq")
                nc.sync.dma_start(out=xq[:], in_=xtq[:, j * P:(j + 1) * P])
                psq = mmd.tile([P, D], F32, tag="pd2")
                nc.tensor.matmul(out=psq[:], lhsT=xq[:],
                                 rhs=c_wqt[:], start=True, stop=True)
                qxj = c_qx[:, j * D:(j + 1) * D]
                if has_bq:
                    nc.vector.tensor_add(qxj, psq[:], c_bqr[:])
                else:
                    nc.scalar.copy(qxj, psq[:])
                ps_seg = mmseg.tile([P, 3, D + H], F32, tag="seg")
                xsl = slab.tile([D, ST, P], BF16, tag="xe")
                nc.sync.dma_start(
                    out=xsl[:, :stn, :],
                    in_=xet[:, ti * P:(ti + stn) * P].rearrange(
                        "p (c n) -> p c n", c=stn))

                # P1: per-edge K|V projection, PSUM -> SBUF (Act copies)
                kvs = kvsp.tile([P, ST, 2 * D], BF16, tag="kvs")
                for g0 in range(0, stn, G):
                    gw = min(G, stn - g0)
                    kvp = mmb.tile([P, G, 2 * D], F32, tag="pb")
                    for cc in range(gw):
                        nc.tensor.matmul(out=kvp[:, cc, :],
                                         lhsT=xsl[:, g0 + cc, :],
                                         rhs=c_wkvt[:], start=True, stop=True)
                    if has_bkv:
                        b0 = c_bkvr[:]
                        b_b = bass.AP(b0.tensor, b0.offset,
                                      [b0.ap[0], [0, gw], [1, 2 * D]])
                        nc.vector.tensor_tensor(
                            out=kvs[:, g0:g0 + gw, :], in0=kvp[:, :gw, :],
                            in1=b_b, op=mybir.AluOpType.add)
                    else:
                        nc.scalar.copy(
                            kvs[:, g0:g0 + gw, :].rearrange("p c n -> p (c n)"),
                            kvp[:, :gw, :].rearrange("p c n -> p (c n)"))

                # P2: whole-slot fused DVE/Act ops
                qk = sct.tile([P, ST, D], BF16, tag="qk")
                q_b = bass.AP(qxj.tensor, qxj.offset,
                              [qxj.ap[0], [0, stn], [1, D]])
                veng = nc.vector
                veng.tensor_tensor(
                    out=qk[:, :stn, :], in0=kvs[:, :stn, :D],
                    in1=q_b, op=mybir.AluOpType.mult)
                s8 = sct.tile([P, ST, H, 8], BF16, tag="s8")
                qk4 = qk[:, :stn, :].rearrange("p c (h x) -> p c h x", h=H)
                nc.vector.tensor_tensor(
                    out=s8[:, :stn, :, :], in0=qk4[:, :, :, :8],
                    in1=qk4[:, :, :, 8:], op=mybir.AluOpType.add)
                s4 = sct.tile([P, ST, H, 4], BF16, tag="s4")
                nc.vector.tensor_tensor(
                    out=s4[:, :stn, :, :], in0=s8[:, :stn, :, :4],
                    in1=s8[:, :stn, :, 4:], op=mybir.AluOpType.add)
                s2 = sct.tile([P, ST, H, 2], BF16, tag="s2")
                nc.vector.tensor_tensor(
                    out=s2[:, :stn, :, :], in0=s4[:, :stn, :, :2],
                    in1=s4[:, :stn, :, 2:], op=mybir.AluOpType.add)
                sc = sct.tile([P, ST, H], BF16, tag="sc")
                nc.vector.tensor_tensor(
                    out=sc[:, :stn, :], in0=s2[:, :stn, :, 0],
                    in1=s2[:, :stn, :, 1], op=mybir.AluOpType.add)
                rhs = edges.tile([P, ST, D + H], BF16, tag="rhs")
                nc.scalar.activation(
                    out=rhs[:, :stn, D:], in_=sc[:, :stn, :],
                    func=mybir.ActivationFunctionType.Exp, scale=scale)
                r0 = rhs[:]
                ex_b = bass.AP(r0.tensor, r0.offset + D,
                               [r0.ap[0], [D + H, stn], [1, H], [0, HD]])
                veng.tensor_tensor(
                    out=rhs[:, :stn, :D].rearrange("p c (h x) -> p c h x",
                                                   h=H),
                    in0=kvs[:, :stn, D:].rearrange("p c (h x) -> p c h x",
                                                   h=H),
                    in1=ex_b, op=mybir.AluOpType.mult)
                # DVE pair-fold halves the PE seg-matmul count
                fold = edges.tile([P, (ST + 1) // 2, D + H], BF16, tag="fold")
                npair = stn // 2
                if npair:
                    r3 = rhs[:]
                    ev = bass.AP(r3.tensor, r3.offset,
                                 [r3.ap[0], [2 * (D + H), npair], [1, D + H]])
                    od = bass.AP(r3.tensor, r3.offset + (D + H),
                                 [r3.ap[0], [2 * (D + H), npair], [1, D + H]])
                    nc.vector.tensor_tensor(out=fold[:, :npair, :], in0=ev,
                                            in1=od, op=mybir.AluOpType.add)
                odd = stn % 2
                for c0 in range(0, npair, 3):
                    cw = min(3, npair - c0)
                    nc.tensor.matmul(
                        out=ps_seg[:, :cw, :].rearrange("p c n -> p (c n)"),
                        lhsT=c_ident[:],
                        rhs=fold[:, c0:c0 + cw, :].rearrange(
                            "p c n -> p (c n)"),
                        start=(c0 == 0), stop=(not odd and c0 + 3 >= npair))
                if odd:
                    nc.tensor.matmul(
                        out=ps_seg[:, 0, :], lhsT=c_ident[:],
                        rhs=rhs[:, stn - 1, :],
                        start=(npair == 0), stop=True)
                ti += stn

                # ---- slot epilogue (fold the 2 seg accumulators)
                nd = blk.tile([P, D + H], F32, tag="nd")
                nc.vector.tensor_scalar_add(nd[:], ps_seg[:, 0, :], 0.0)
                for acc in range(1, min(3, max(stn // 2, 1))):
                    nc.vector.tensor_tensor(out=nd[:], in0=nd[:],
                                            in1=ps_seg[:, acc, :],
                                            op=mybir.AluOpType.add)
                den = blk.tile([P, H], F32, tag="den")
                nc.vector.tensor_scalar(
                    out=den[:], in0=nd[:, D:], scalar1=c_dcnt[:, j:j + 1],
                    scalar2=DEN_GUARD, op0=mybir.AluOpType.subtract,
                    op1=mybir.AluOpType.add)
                rec = blk.tile([P, H], F32, tag="rec")
                nc.vector.reciprocal(rec[:], den[:])
                rr = rec[:]
                rec_b = bass.AP(rr.tensor, rr.offset,
                                [rr.ap[0], [1, H], [0, HD]])
                if j % CH == 0:
                    attn_ch = epi.tile([P, SC], BF16, tag="attnch")
                jo = (j % CH) * D
                nc.vector.tensor_tensor(
                    out=attn_ch[:, jo:jo + D].rearrange(
                        "p (h x) -> p h x", h=H),
                    in0=nd[:, :D].rearrange("p (h x) -> p h x", h=H),
                    in1=rec_b, op=mybir.AluOpType.mult)
                if j % CH == CH - 1:
                    emit_chunk(j - CH + 1, attn_ch)

    nc.compile()
    return nc


# ----------------------------------------------------------------------------
# Runner / public API
# ----------------------------------------------------------------------------

_LAST = {}
_CACHE = {}


def _get_program(key, *args):
    if key not in _CACHE:
        _CACHE[key] = build_program(*args)
    return _CACHE[key]


def kernel(X, attn_window, Wq, bq, Wk, bk, Wv, bv, Wo, bo, g1, b1, g2, b2):
    n_nodes, D = X.shape
    H = 8
    sch, in_maps, flags = _prep_inputs(X, attn_window, Wq, bq, Wk, bk, Wv, bv,
                                       Wo, bo, g1, b1, g2, b2)
    key = (sch["n_pad"], sch["slots"], sch["T"], tuple(sch["slot_tiles"]),
           D, flags)
    nc = _get_program(key, sch["n_pad"], sch["slots"], sch["T"],
                      sch["slot_tiles"], D, H, flags)
    _LAST.update(nc=nc, sch=sch, in_maps=in_maps)
    res = run_bass_kernel_spmd(nc, in_maps, core_ids=list(range(NCORES)))
    out = np.empty((n_nodes, D), dtype=np.float32)
    for c in range(NCORES):
        oc = np.asarray(res.results[c]["OUT"]).astype(np.float32)
        ids = sch["nodeids"][c]
        valid = ids < n_nodes
        out[ids[valid]] = oc[valid]
    return out


# revision 27
# speedup vs baseline: 1.0532x; 1.0532x over previous
"""Trainium2 Bass kernel for NaiveKHopGraphAttention (lane + edge-slab).

Strategy (no collectives, no device-side gather):
  - Host (integer index work only): sort nodes by degree, group into
    128-node blocks of near-equal degree, assign blocks to (core, slot)
    with SPMD-uniform per-slot tile counts. Tile t of a slot holds the
    t-th edge of each of the block's 128 nodes (lane layout). The host
    ships, per core, the transposed X rows of each lane's dst node
    (XET = X[dst].T, bf16) - pure indexing of the input, no float math.
    Pad lanes point at a zero row.
  - Device phases:
      A: Q projection -> SBUF-resident qx (bf16, node-partition).
      C: per slot, per 4-tile group: PE projects K|V per edge
         (lhsT = slab tile, rhs = [Wk.T|Wv.T]) into PSUM; rotating
         engine copies PSUM->SBUF bf16; DVE qk = K*q_bcast; Pool
         per-head score reduce; Act exp (strided into rhs[:, :, D:]);
         DVE wv = V*ex_bcast; PE identity-matmul accumulates each
         tile's [wv | ex] into a per-slot PSUM [num | den] (lane p is
         node p, so the segment sum is a plain copy-accumulate).
         Dummy lanes contribute ex=1, V=0; the denominator is fixed
         exactly with a host-computed dummy count. Then divide and
         store the attn row block into SBUF.
      D: batched epilogue: LayerNorm1 across all slots in wide DVE ops
         (g1 folded into Wo), per-slot PE transpose + out-projection,
         batched LayerNorm2, single output DMA (bf16; host converts).
"""

import sys

if "/opt/trn_rl_repo" not in sys.path:
    sys.path.insert(0, "/opt/trn_rl_repo")

import ml_dtypes
import numpy as np

BF16NP = ml_dtypes.bfloat16

import concourse.bacc as bacc
import concourse.bass as bass
import concourse.mybir as mybir
import concourse.tile as tile
from concourse.bass_utils import run_bass_kernel_spmd

F32 = mybir.dt.float32
BF16 = mybir.dt.bfloat16
I32 = mybir.dt.int32

NCORES = 8
P = 128
EPS = 1e-5
DEN_GUARD = 1e-30
G = 4        # edge tiles per compute group


# ----------------------------------------------------------------------------
# Host-side preprocessing
# ----------------------------------------------------------------------------

def _schedule(src, dst, n_nodes):
    n_blocks = -(-n_nodes // P)
    n_blocks = -(-n_blocks // NCORES) * NCORES
    n_pad = n_blocks * P
    slots = n_blocks // NCORES
    assert n_pad > n_nodes, "need at least one zero pad node"

    deg = np.bincount(src, minlength=n_pad).astype(np.int64)

    perm = np.argsort(-deg, kind="stable")
    pos = np.empty(n_pad, dtype=np.int64)
    pos[perm] = np.arange(n_pad)
    blk = pos // P
    p_of = pos % P
    j_of = blk // NCORES
    c_of = blk % NCORES

    # per-slot tile count = max degree across the slot's 8 blocks
    deg_blk = deg[perm].reshape(n_blocks, P).max(axis=1)
    stn = deg_blk.reshape(slots, NCORES).max(axis=1)
    stn = np.maximum(stn, 1)
    tile_off = np.zeros(slots + 1, dtype=np.int64)
    np.cumsum(stn, out=tile_off[1:])
    T = int(tile_off[-1])

    # lane dst ids: LID[c, p, tile_off[j]+t] = dst of node's t-th edge
    order = np.argsort(src, kind="stable")
    src_s = src[order]
    dst_s = dst[order]
    cnt = np.bincount(src, minlength=n_pad)
    noff = np.zeros(n_pad + 1, dtype=np.int64)
    np.cumsum(cnt, out=noff[1:])
    rank = np.arange(len(order)) - noff[src_s]

    lid = np.full((NCORES, P, T), n_pad - 1, dtype=np.int64)  # pad: zero row
    cs, ps, js = c_of[src_s], p_of[src_s], j_of[src_s]
    lid[cs, ps, tile_off[js] + rank] = dst_s

    dcnt = np.empty((NCORES, P, slots), dtype=np.float32)
    dcnt[c_of, p_of, j_of] = (stn[j_of] - deg).astype(np.float32)

    nodeids = np.empty((NCORES, slots * P), dtype=np.int64)
    nodeids[c_of, j_of * P + p_of] = np.arange(n_pad)

    return {
        "n_pad": n_pad,
        "slots": slots,
        "T": T,
        "slot_tiles": [int(x) for x in stn],
        "lid": lid,
        "dcnt": dcnt,
        "nodeids": nodeids,
    }


def _prep_inputs(X, attn_window, Wq, bq, Wk, bk, Wv, bv, Wo, bo, g1, b1, g2, b2):
    n_nodes, D = X.shape
    src = np.asarray(attn_window[0]).astype(np.int64)
    dst = np.asarray(attn_window[1]).astype(np.int64)
    sch = _schedule(src, dst, n_nodes)
    n_pad, slots, T = sch["n_pad"], sch["slots"], sch["T"]

    Xp = np.zeros((n_pad, D), dtype=np.float32)
    Xp[:n_nodes] = np.asarray(X, np.float32)
    XTb = np.ascontiguousarray(Xp.T).astype(BF16NP)  # [D, n_pad] bf16

    WoT = np.asarray(Wo, np.float32).T
    Wo2T = np.ascontiguousarray(WoT * np.asarray(g1, np.float32)[:, None])
    BO2 = (np.asarray(b1, np.float32) @ WoT + np.asarray(bo, np.float32))[None, :]

    has_bkv = bool(np.any(np.asarray(bk) != 0) or np.any(np.asarray(bv) != 0))
    has_bq = bool(np.any(np.asarray(bq) != 0))
    has_bo2 = bool(np.any(BO2 != 0))
    has_g2 = bool(np.any(np.asarray(g2) != 1))
    has_b2 = bool(np.any(np.asarray(b2) != 0))
    flags = (has_bkv, has_bq, has_bo2, has_g2, has_b2)

    common = {
        "WKVT": np.ascontiguousarray(
            np.concatenate([np.asarray(Wk, np.float32).T,
                            np.asarray(Wv, np.float32).T], axis=1)
        ).astype(BF16NP),
        "WQT": np.ascontiguousarray(np.asarray(Wq, np.float32).T).astype(BF16NP),
        "WO2T": Wo2T.astype(BF16NP),
        "IDENT": np.eye(P, dtype=np.float32).astype(BF16NP),
    }
    if has_bkv:
        common["BKVR"] = np.broadcast_to(
            np.concatenate([np.asarray(bk, np.float32),
                            np.asarray(bv, np.float32)])[None, :],
            (P, 2 * D)).copy()
    if has_bq:
        common["BQR"] = np.broadcast_to(
            np.asarray(bq, np.float32)[None, :], (P, D)).copy()
    if has_bo2:
        common["BO2R"] = np.broadcast_to(BO2, (P, D)).copy()
    if has_g2:
        common["G2R"] = np.broadcast_to(
            np.asarray(g2, np.float32)[None, :], (P, D)).astype(BF16NP).copy()
    if has_b2:
        common["B2R"] = np.broadcast_to(
            np.asarray(b2, np.float32)[None, :], (P, D)).astype(BF16NP).copy()

    in_maps = []
    for c in range(NCORES):
        m = dict(common)
        m["XTQ"] = np.ascontiguousarray(
            Xp[sch["nodeids"][c]].T).astype(BF16NP)
        # edge slab: X rows of each lane's dst, transposed [D, T*P].
        # lid[c] is [P, T] (lane p, tile t); lane order in a tile is p.
        lid_flat = sch["lid"][c].T.ravel()               # t-major, then p
        m["XET"] = np.ascontiguousarray(XTb[:, lid_flat])
        m["DCNT"] = np.ascontiguousarray(sch["dcnt"][c])
        in_maps.append(m)
    return sch, in_maps, flags


# ----------------------------------------------------------------------------
# Device kernel
# ----------------------------------------------------------------------------

def _newton_rsqrt(nc, pool, v_ap, width, tag, iters=2):
    """rstd = 1/sqrt(v) on DVE only. v_ap: [P, width] f32 (eps added)."""
    y = pool.tile([P, width], F32, tag=tag + "_y")
    u = pool.tile([P, width], I32, tag=tag + "_u")
    nc.vector.tensor_scalar(
        out=u[:], in0=v_ap.bitcast(I32), scalar1=1, scalar2=None,
        op0=mybir.AluOpType.arith_shift_right)
    nc.vector.tensor_scalar(
        out=y[:].bitcast(I32), in0=u[:], scalar1=0x5F3759DF, scalar2=-1,
        op0=mybir.AluOpType.subtract, op1=mybir.AluOpType.mult)
    t = pool.tile([P, width], F32, tag=tag + "_t")
    for _ in range(iters):
        nc.vector.tensor_mul(t[:], y[:], y[:])
        nc.vector.tensor_mul(t[:], t[:], v_ap)
        nc.vector.tensor_scalar(
            out=t[:], in0=t[:], scalar1=-0.5, scalar2=1.5,
            op0=mybir.AluOpType.mult, op1=mybir.AluOpType.add)
        nc.vector.tensor_mul(y[:], y[:], t[:])
    return y


def build_program(n_pad, slots, T, slot_tiles, D=128, H=8,
                  flags=(False, False, False, False, False)):
    has_bkv, has_bq, has_bo2, has_g2, has_b2 = flags
    HD = D // H
    scale = 1.0 / np.sqrt(HD)
    S = slots * D

    nc = bacc.Bacc("TRN2", target_bir_lowering=False, debug=False,
                   num_devices=NCORES)

    xet = nc.dram_tensor("XET", [D, T * P], BF16, kind="ExternalInput").ap()
    xtq = nc.dram_tensor("XTQ", [D, slots * P], BF16, kind="ExternalInput").ap()
    wkvt = nc.dram_tensor("WKVT", [D, 2 * D], BF16, kind="ExternalInput").ap()
    wqt = nc.dram_tensor("WQT", [D, D], BF16, kind="ExternalInput").ap()
    wo2t = nc.dram_tensor("WO2T", [D, D], BF16, kind="ExternalInput").ap()
    ident_in = nc.dram_tensor("IDENT", [P, P], BF16, kind="ExternalInput").ap()
    dcnt_in = nc.dram_tensor("DCNT", [P, slots], F32, kind="ExternalInput").ap()
    if has_bkv:
        bkvr = nc.dram_tensor("BKVR", [P, 2 * D], F32, kind="ExternalInput").ap()
    if has_bq:
        bqr = nc.dram_tensor("BQR", [P, D], F32, kind="ExternalInput").ap()
    if has_bo2:
        bo2r = nc.dram_tensor("BO2R", [P, D], F32, kind="ExternalInput").ap()
    if has_g2:
        g2r = nc.dram_tensor("G2R", [P, D], BF16, kind="ExternalInput").ap()
    if has_b2:
        b2r = nc.dram_tensor("B2R", [P, D], BF16, kind="ExternalInput").ap()
    out = nc.dram_tensor("OUT", [slots * P, D], BF16, kind="ExternalOutput").ap()

    with tile.TileContext(nc) as tc:
        with (
            tc.tile_pool(name="consts", bufs=1) as consts,
            tc.tile_pool(name="slab", bufs=2) as slab,
            tc.tile_pool(name="kvsp", bufs=3) as kvsp,
            tc.tile_pool(name="edges", bufs=2) as edges,
            tc.tile_pool(name="sct", bufs=3) as sct,
            tc.tile_pool(name="blk", bufs=2) as blk,
            tc.tile_pool(name="epi", bufs=2) as epi,
            tc.tile_pool(name="mmb", bufs=2, space="PSUM") as mmb,
            tc.tile_pool(name="mmseg", bufs=2, space="PSUM") as mmseg,
            tc.tile_pool(name="mmd", bufs=1, space="PSUM") as mmd,
        ):
            # ---- constants
            c_wkvt = consts.tile([D, 2 * D], BF16, tag="wkvt")
            nc.sync.dma_start(out=c_wkvt[:], in_=wkvt[:])
            c_wqt = consts.tile([D, D], BF16, tag="wqt")
            nc.sync.dma_start(out=c_wqt[:], in_=wqt[:])
            c_wo2t = consts.tile([D, D], BF16, tag="wo2t")
            nc.sync.dma_start(out=c_wo2t[:], in_=wo2t[:])
            c_ident = consts.tile([P, P], BF16, tag="ident")
            nc.sync.dma_start(out=c_ident[:], in_=ident_in[:])
            c_dcnt = consts.tile([P, slots], F32, tag="dcnt")
            nc.sync.dma_start(out=c_dcnt[:], in_=dcnt_in[:])
            if has_bkv:
                c_bkvr = consts.tile([P, 2 * D], F32, tag="bkvr")
                nc.sync.dma_start(out=c_bkvr[:], in_=bkvr[:])
            if has_bq:
                c_bqr = consts.tile([P, D], F32, tag="bqr")
                nc.sync.dma_start(out=c_bqr[:], in_=bqr[:])
            if has_bo2:
                c_bo2r = consts.tile([P, D], F32, tag="bo2r")
                nc.sync.dma_start(out=c_bo2r[:], in_=bo2r[:])
            if has_g2:
                c_g2 = consts.tile([P, D], BF16, tag="g2")
                nc.sync.dma_start(out=c_g2[:], in_=g2r[:])
            if has_b2:
                c_b2 = consts.tile([P, D], BF16, tag="b2")
                nc.sync.dma_start(out=c_b2[:], in_=b2r[:])
            c_qx = consts.tile([P, slots * D], BF16, tag="qx")

            # ---- Phase C: edge stage per slot (two passes, big fused ops)
            # The epilogue (LN1 -> out-proj -> LN2 -> store) runs in
            # CH-slot chunks interleaved into the slot loop so it hides
            # under edge-phase compute instead of forming a serial tail.
            ST = max(slot_tiles)
            CH = 7
            assert slots % CH == 0
            SC = CH * D

            def layer_norm_chunk(x_flat, out_flat, tagp="ln"):
                x3 = x_flat.rearrange("p (j d) -> p j d", j=CH)
                s1 = epi.tile([P, CH], F32, tag=tagp + "s1")
                nc.vector.tensor_reduce(out=s1[:], in_=x3,
                                        axis=mybir.AxisListType.X,
                                        op=mybir.AluOpType.add)
                nm = epi.tile([P, CH], F32, tag=tagp + "nm")
                nc.vector.tensor_scalar(
                    out=nm[:], in0=s1[:], scalar1=-1.0 / D, scalar2=None,
                    op0=mybir.AluOpType.mult)
                sq = epi.tile([P, SC], BF16, tag=tagp + "sq")
                nc.vector.tensor_tensor(out=sq[:], in0=x_flat, in1=x_flat,
                                        op=mybir.AluOpType.mult)
                s2 = epi.tile([P, CH], F32, tag=tagp + "s2")
                nc.vector.tensor_reduce(
                    out=s2[:], in_=sq[:].rearrange("p (j d) -> p j d", j=CH),
                    axis=mybir.AxisListType.X, op=mybir.AluOpType.add)
                ve = epi.tile([P, CH], F32, tag=tagp + "ve")
                nc.vector.tensor_mul(ve[:], nm[:], nm[:])
                nc.vector.scalar_tensor_tensor(
                    out=ve[:], in0=s2[:], scalar=1.0 / D, in1=ve[:],
                    op0=mybir.AluOpType.mult, op1=mybir.AluOpType.subtract)
                nc.vector.tensor_scalar(
                    out=ve[:], in0=ve[:], scalar1=EPS, scalar2=None,
                    op0=mybir.AluOpType.add)
                rstd = _newton_rsqrt(nc, epi, ve[:], CH, tagp + "r")
                nm0, rs0 = nm[:], rstd[:]
                nm_b = bass.AP(nm0.tensor, nm0.offset,
                               [nm0.ap[0], [1, CH], [0, D]])
                rs_b = bass.AP(rs0.tensor, rs0.offset,
                               [rs0.ap[0], [1, CH], [0, D]])
                xm = epi.tile([P, SC], BF16, tag=tagp + "xm")
                nc.vector.tensor_tensor(
                    out=xm[:].rearrange("p (j d) -> p j d", j=CH),
                    in0=x3, in1=nm_b, op=mybir.AluOpType.add)
                nc.vector.tensor_tensor(
                    out=out_flat.rearrange("p (j d) -> p j d", j=CH),
                    in0=xm[:].rearrange("p (j d) -> p j d", j=CH),
                    in1=rs_b, op=mybir.AluOpType.mult)

            def emit_chunk(j0, attn_ch):
                xh = epi.tile([P, SC], BF16, tag="xh")
                layer_norm_chunk(attn_ch[:], xh[:])
                o2 = epi.tile([P, SC], BF16, tag="o2")
                for jj in range(CH):
                    pst = mmd.tile([P, D], BF16, tag="pd")
                    nc.tensor.transpose(out=pst[:],
                                        in_=xh[:, jj * D:(jj + 1) * D],
                                        identity=c_ident[:])
                    lnt = blk.tile([P, D], BF16, tag="lnt")
                    nc.scalar.copy(lnt[:], pst[:])
                    ps2 = mmd.tile([P, D], F32, tag="pd2")
                    nc.tensor.matmul(out=ps2[:], lhsT=lnt[:], rhs=c_wo2t[:],
                                     start=True, stop=True)
                    if has_bo2:
                        nc.vector.tensor_add(o2[:, jj * D:(jj + 1) * D],
                                             ps2[:], c_bo2r[:])
                    elif jj % 2 == 0:
                        nc.scalar.copy(o2[:, jj * D:(jj + 1) * D], ps2[:])
                    else:
                        nc.vector.tensor_copy(o2[:, jj * D:(jj + 1) * D],
                                              ps2[:])
                fin = epi.tile([P, SC], BF16, tag="xh")  # reuse xh buffer
                layer_norm_chunk(o2[:], fin[:])
                fin_ap = fin[:]
                if has_g2:
                    gg = c_g2[:]
                    g_b = bass.AP(gg.tensor, gg.offset,
                                  [gg.ap[0], [0, CH], [1, D]])
                    fg = epi.tile([P, SC], BF16, tag="lnsq")
                    nc.vector.tensor_tensor(
                        out=fg[:].rearrange("p (j d) -> p j d", j=CH),
                        in0=fin_ap.rearrange("p (j d) -> p j d", j=CH),
                        in1=g_b, op=mybir.AluOpType.mult)
                    fin_ap = fg[:]
                if has_b2:
                    bb = c_b2[:]
                    b_b = bass.AP(bb.tensor, bb.offset,
                                  [bb.ap[0], [0, CH], [1, D]])
                    fb = epi.tile([P, SC], BF16, tag="lnxm")
                    nc.vector.tensor_tensor(
                        out=fb[:].rearrange("p (j d) -> p j d", j=CH),
                        in0=fin_ap.rearrange("p (j d) -> p j d", j=CH),
                        in1=b_b, op=mybir.AluOpType.add)
                    fin_ap = fb[:]
                nc.sync.dma_start(
                    out=out[j0 * P:(j0 + CH) * P, :].rearrange(
                        "(j p) d -> p j d", p=P),
                    in_=fin_ap.rearrange("p (j d) -> p j d", j=CH))

            ti = 0
            attn_ch = None
            for j in range(slots):
                stn = slot_tiles[j]
                # Q projection for this slot (interleaved with edge work)
                xq = slab.tile([D, P], BF16, tag="xq")
                nc.sync.dma_start(out=xq[:], in_=xtq[:, j * P:(j + 1) * P])
                psq = mmd.tile([P, D], F32, tag="pd2")
                nc.tensor.matmul(out=psq[:], lhsT=xq[:],
                                 rhs=c_wqt[:], start=True, stop=True)
                qxj = c_qx[:, j * D:(j + 1) * D]
                if has_bq:
                    nc.vector.tensor_add(qxj, psq[:], c_bqr[:])
                else:
                    nc.scalar.copy(qxj, psq[:])
                ps_seg = mmseg.tile([P, 3, D + H], F32, tag="seg")
                xsl = slab.tile([D, ST, P], BF16, tag="xe")
                nc.sync.dma_start(
                    out=xsl[:, :stn, :],
                    in_=xet[:, ti * P:(ti + stn) * P].rearrange(
                        "p (c n) -> p c n", c=stn))

                # P1: per-edge K|V projection, PSUM -> SBUF (Act copies)
                kvs = kvsp.tile([P, ST, 2 * D], BF16, tag="kvs")
                for g0 in range(0, stn, G):
                    gw = min(G, stn - g0)
                    kvp = mmb.tile([P, G, 2 * D], F32, tag="pb")
                    for cc in range(gw):
                        nc.tensor.matmul(out=kvp[:, cc, :],
                                         lhsT=xsl[:, g0 + cc, :],
                                         rhs=c_wkvt[:], start=True, stop=True)
                    if has_bkv:
                        b0 = c_bkvr[:]
                        b_b = bass.AP(b0.tensor, b0.offset,
                                      [b0.ap[0], [0, gw], [1, 2 * D]])
                        nc.vector.tensor_tensor(
                            out=kvs[:, g0:g0 + gw, :], in0=kvp[:, :gw, :],
                            in1=b_b, op=mybir.AluOpType.add)
                    else:
                        nc.scalar.copy(
                            kvs[:, g0:g0 + gw, :].rearrange("p c n -> p (c n)"),
                            kvp[:, :gw, :].rearrange("p c n -> p (c n)"))

                # P2: whole-slot fused DVE/Act ops
                qk = sct.tile([P, ST, D], BF16, tag="qk")
                q_b = bass.AP(qxj.tensor, qxj.offset,
                              [qxj.ap[0], [0, stn], [1, D]])
                veng = nc.vector
                veng.tensor_tensor(
                    out=qk[:, :stn, :], in0=kvs[:, :stn, :D],
                    in1=q_b, op=mybir.AluOpType.mult)
                s8 = sct.tile([P, ST, H, 8], BF16, tag="s8")
                qk4 = qk[:, :stn, :].rearrange("p c (h x) -> p c h x", h=H)
                nc.vector.tensor_tensor(
                    out=s8[:, :stn, :, :], in0=qk4[:, :, :, :8],
                    in1=qk4[:, :, :, 8:], op=mybir.AluOpType.add)
                s4 = sct.tile([P, ST, H, 4], BF16, tag="s4")
                nc.vector.tensor_tensor(
                    out=s4[:, :stn, :, :], in0=s8[:, :stn, :, :4],
                    in1=s8[:, :stn, :, 4:], op=mybir.AluOpType.add)
                s2 = sct.tile([P, ST, H, 2], BF16, tag="s2")
                nc.vector.tensor_tensor(
                    out=s2[:, :stn, :, :], in0=s4[:, :stn, :, :2],
                    in1=s4[:, :stn, :, 2:], op=mybir.AluOpType.add)
                sc = sct.tile([P, ST, H], BF16, tag="sc")
                nc.vector.tensor_tensor(
                    out=sc[:, :stn, :], in0=s2[:, :stn, :, 0],
                    in1=s2[:, :stn, :, 1], op=mybir.AluOpType.add)
                rhs = edges.tile([P, ST, D + H], BF16, tag="rhs")
                nc.scalar.activation(
                    out=rhs[:, :stn, D:], in_=sc[:, :stn, :],
                    func=mybir.ActivationFunctionType.Exp, scale=scale)
                r0 = rhs[:]
                ex_b = bass.AP(r0.tensor, r0.offset + D,
                               [r0.ap[0], [D + H, stn], [1, H], [0, HD]])
                veng.tensor_tensor(
                    out=rhs[:, :stn, :D].rearrange("p c (h x) -> p c h x",
                                                   h=H),
                    in0=kvs[:, :stn, D:].rearrange("p c (h x) -> p c h x",
                                                   h=H),
                    in1=ex_b, op=mybir.AluOpType.mult)
                for c0 in range(0, stn, 3):
                    cw = min(3, stn - c0)
                    nc.tensor.matmul(
                        out=ps_seg[:, :cw, :].rearrange("p c n -> p (c n)"),
                        lhsT=c_ident[:],
                        rhs=rhs[:, c0:c0 + cw, :].rearrange("p c n -> p (c n)"),
                        start=(c0 == 0), stop=(c0 + 3 >= stn))
                ti += stn

                # ---- slot epilogue (fold the 2 seg accumulators)
                nd = blk.tile([P, D + H], F32, tag="nd")
                nc.vector.tensor_scalar_add(nd[:], ps_seg[:, 0, :], 0.0)
                for acc in range(1, min(3, stn)):
                    nc.vector.tensor_tensor(out=nd[:], in0=nd[:],
                                            in1=ps_seg[:, acc, :],
                                            op=mybir.AluOpType.add)
                den = blk.tile([P, H], F32, tag="den")
                nc.vector.tensor_scalar(
                    out=den[:], in0=nd[:, D:], scalar1=c_dcnt[:, j:j + 1],
                    scalar2=DEN_GUARD, op0=mybir.AluOpType.subtract,
                    op1=mybir.AluOpType.add)
                rec = blk.tile([P, H], F32, tag="rec")
                nc.vector.reciprocal(rec[:], den[:])
                rr = rec[:]
                rec_b = bass.AP(rr.tensor, rr.offset,
                                [rr.ap[0], [1, H], [0, HD]])
                if j % CH == 0:
                    attn_ch = epi.tile([P, SC], BF16, tag="attnch")
                jo = (j % CH) * D
                nc.vector.tensor_tensor(
                    out=attn_ch[:, jo:jo + D].rearrange(
                        "p (h x) -> p h x", h=H),
                    in0=nd[:, :D].rearrange("p (h x) -> p h x", h=H),
                    in1=rec_b, op=mybir.AluOpType.mult)
                if j % CH == CH - 1:
                    emit_chunk(j - CH + 1, attn_ch)

    nc.compile()
    return nc


# ----------------------------------------------------------------------------
# Runner / public API
# ----------------------------------------------------------------------------

_LAST = {}
_CACHE = {}


def _get_program(key, *args):
    if key not in _CACHE:
        _CACHE[key] = build_program(*args)
    return _CACHE[key]


def kernel(X, attn_window, Wq, bq, Wk, bk, Wv, bv, Wo, bo, g1, b1, g2, b2):
    n_nodes, D = X.shape
    H = 8
    sch, in_maps, flags = _prep_inputs(X, attn_window, Wq, bq, Wk, bk, Wv, bv,
                                       Wo, bo, g1, b1, g2, b2)
    key = (sch["n_pad"], sch["slots"], sch["T"], tuple(sch["slot_tiles"]),
           D, flags)
    nc = _get_program(key, sch["n_pad"], sch["slots"], sch["T"],
                      sch["slot_tiles"], D, H, flags)
    _LAST.update(nc=nc, sch=sch, in_maps=in_maps)
    res = run_bass_kernel_spmd(nc, in_maps, core_ids=list(range(NCORES)))
    out = np.empty((n_nodes, D), dtype=np.float32)
    for c in range(NCORES):
        oc = np.asarray(res.results[c]["OUT"]).astype(np.float32)
        ids = sch["nodeids"][c]
        valid = ids < n_nodes
        out[ids[valid]] = oc[valid]
    return out
